# revision 12
# baseline (speedup 1.0000x reference)
"""Causal multi-head attention block (qkv -> causal softmax(qk^T/sqrt(d)) v -> proj)
for Trainium2, batch-sharded across 8 NeuronCores (1 batch element per core).

Self-contained: hardcodes shapes B=8, S=1024, NX=1024, H=16, D=64.

Per-core dataflow (all matmuls fp32r = full-rate reduced-precision fp32):
  x [S,NX] --PE transpose--> xT [NX,S]
  v  = xT.T @ w_v   (+b)    -> v  [S,NX]   (natural)
  qkT = w_qk.T...: matmul(lhsT=w_attn[:, :2048], rhs=xT) -> qT,kT [2048,S] (transposed)
  per head pair (2 heads/128 partitions):
    sT[k,q] = matmul(lhsT=kT_h[64,k-chunk], rhs=qT_h[64,q-chunk])   row-tiled x2 heads
    causal:   fully-masked tiles skipped; diagonal 128x128 blocks get
              += I.T @ T  (T = -30000 below diagonal) accumulated in PSUM
    wT = Exp(sT/8)                       (ScalarE, PSUM->SBUF fp32r)
    aT_u = matmul(lhsT=v_h[k,64], rhs=wT) col-tiled x2 heads -> [128(d),q]
    sum  = matmul(lhsT=ones[128,1], rhs=wT) col-tiled x2 heads (M=1 each)
    aT   = aT_u * broadcast(1/sum)       (VectorE eviction, fp32r)
  out = matmul(lhsT=aT, rhs=w_proj) (+b) -> [S,NX]
"""

import sys

if "/opt/trn_rl_repo" not in sys.path:
    sys.path.insert(0, "/opt/trn_rl_repo")

import numpy as np
from contextlib import ExitStack

import concourse.bass as bass
import concourse.mybir as mybir
import concourse.tile as tile
from concourse import bacc
from concourse.bass_utils import run_bass_kernel_spmd
from concourse.masks import make_identity

F32 = mybir.dt.float32
F32R = mybir.dt.float32r
BF16 = mybir.dt.bfloat16
EXP = mybir.ActivationFunctionType.Exp

P = 128
S = 1024
NX = 1024
H = 16
D = 64
NT = S // P            # 8 partition tiles along S or NX
NQC = 2                # 512-wide q windows
QW = 512
NPAIR = H // 2         # 8 head pairs
SCALE = 0.125          # 1/sqrt(64)
NEGBIG = -30000.0      # pre-scale mask add; exp(SCALE*NEGBIG) == 0 in fp32


def build_nc():
    nc = bacc.Bacc("TRN2", target_bir_lowering=False, debug=False)

    x_d = nc.dram_tensor("x", [S, NX], F32, kind="ExternalInput").ap()
    wattn_d = nc.dram_tensor("w_attn", [NX, 3 * NX], F32R, kind="ExternalInput").ap()
    battn_d = nc.dram_tensor("b_attn", [3 * NX], F32R, kind="ExternalInput").ap()
    wproj_d = nc.dram_tensor("w_proj", [NX, NX], F32R, kind="ExternalInput").ap()
    bproj_d = nc.dram_tensor("b_proj", [NX], F32R, kind="ExternalInput").ap()
    cst_d = nc.dram_tensor("cst", [P, 2 * NX], F32R, kind="ExternalInput").ap()
    out_d = nc.dram_tensor("out", [S, NX], F32, kind="ExternalOutput").ap()

    with tile.TileContext(nc) as tc, ExitStack() as ctx:
        const = ctx.enter_context(tc.tile_pool(name="const", bufs=1))
        p_xT = ctx.enter_context(tc.tile_pool(name="p_xT", bufs=1))
        p_v = ctx.enter_context(tc.tile_pool(name="p_v", bufs=1))
        p_aT = ctx.enter_context(tc.tile_pool(name="p_aT", bufs=1))

        # ---------- constants ----------
        ident32 = const.tile([P, P], F32)
        make_identity(nc, ident32)
        identb = const.tile([P, P], BF16)
        make_identity(nc, identb)
        tpat = const.tile([P, P], BF16)
        nc.gpsimd.memset(tpat[:], 0.0)
        # tpat[r, c] = 0 if r <= c else NEGBIG   (k on partitions, q on free)
        nc.gpsimd.affine_select(
            out=tpat[:], in_=tpat[:],
            compare_op=mybir.AluOpType.is_ge,
            fill=NEGBIG, base=0,
            pattern=[[1, P]], channel_multiplier=-1,
        )
        ones_row = const.tile([1, QW], F32R)
        nc.sync.dma_start(ones_row[:], cst_d[0:1, 0:QW])
        battn_sb = const.tile([1, 2 * NX], F32R)
        nc.sync.dma_start(battn_sb[:], battn_d.rearrange("(a b) -> a b", a=1)[:, 0 : 2 * NX])

        xT = [p_xT.tile([P, S], F32R, name=f"xT{i}") for i in range(NT)]

        # ---------- phase 1: x transpose ----------
        with tc.tile_pool(name="p_x", bufs=2) as p_x, \
             tc.tile_pool(name="ps_tr", bufs=2, space="PSUM") as ps_tr:
            for st in range(NT):
                xt_in = p_x.tile([P, NX], F32, name="xin")
                nc.sync.dma_start(xt_in[:], x_d[st * P : (st + 1) * P, :])
                for nt in range(NT):
                    pst = ps_tr.tile([P, P], F32, name="trp")
                    nc.tensor.transpose(pst[:], xt_in[:, nt * P : (nt + 1) * P], ident32[:])
                    nc.vector.tensor_copy(xT[nt][:, st * P : (st + 1) * P], pst[:])

        # ---------- phase 2: v = x @ w_v + b_v (natural [S, NX]), augmented with a
        # ones column per head: v_aug[s, h, 0:64] = v head h, v_aug[s, h, 64] = 1
        v = [p_v.tile([P, H, D + 1], F32R, name=f"v{i}") for i in range(NT)]
        with tc.tile_pool(name="p_wv", bufs=1) as p_wv, \
             tc.tile_pool(name="ps_v", bufs=2, space="PSUM") as ps_v:
            battn_v = p_wv.tile([1, NX], F32R, name="battn_v")
            nc.sync.dma_start(
                battn_v[:], battn_d.rearrange("(a b) -> a b", a=1)[:, 2 * NX : 3 * NX]
            )
            wv = [p_wv.tile([P, NX], F32R, name=f"wv{i}") for i in range(NT)]
            for kt in range(NT):
                nc.sync.dma_start(wv[kt][:], wattn_d[kt * P : (kt + 1) * P, 2 * NX : 3 * NX])
            for mt in range(NT):
                nc.sync.dma_start(v[mt][:, :, D : D + 1], cst_d[:, 0:H])
            for mt in range(NT):
                for nn in range(NQC):
                    ps = ps_v.tile([P, QW], F32, name="vps")
                    for kt in range(NT):
                        nc.tensor.matmul(
                            ps[:], xT[kt][:, mt * P : (mt + 1) * P],
                            wv[kt][:, nn * QW : (nn + 1) * QW],
                            start=(kt == 0), stop=False,
                        )
                    nc.tensor.matmul(
                        ps[:], ones_row[:, 0:P],
                        battn_v[:, nn * QW : (nn + 1) * QW],
                        start=False, stop=True,
                    )
                    nc.vector.tensor_copy(
                        v[mt][:, nn * (QW // D) : (nn + 1) * (QW // D), 0:D],
                        ps[:].rearrange("p (h d) -> p h d", d=D),
                    )

        # ---------- phases 3+4: per-pair qkT then attention ----------
        aT = [p_aT.tile([P, S], F32R, name=f"aT{i}") for i in range(NT)]

        attn_ctx = ExitStack()
        p_qk = attn_ctx.enter_context(tc.tile_pool(name="p_qk", bufs=2))
        p_wT = attn_ctx.enter_context(tc.tile_pool(name="p_wT", bufs=1))
        p_wqk = attn_ctx.enter_context(tc.tile_pool(name="p_wqk", bufs=16))
        p_misc = attn_ctx.enter_context(tc.tile_pool(name="p_misc", bufs=3))
        p_bc = attn_ctx.enter_context(tc.tile_pool(name="p_bc", bufs=3))
        wT_A = [p_wT.tile([P, S], F32R, name=f"wTA{i}") for i in range(NT)]
        wT_B = [p_wT.tile([P, S], F32R, name=f"wTB{i}") for i in range(NT)]
        # dead (fully-masked) column ranges zeroed once; exp evictions never touch them
        for ki in range(1, NT):
            nc.sync.dma_start(wT_A[ki][:, 0 : ki * P], cst_d[:, NX : NX + ki * P])
            nc.sync.dma_start(wT_B[ki][:, 0 : ki * P], cst_d[:, NX : NX + ki * P])

        ps_qk = attn_ctx.enter_context(tc.tile_pool(name="ps_qk", bufs=1, space="PSUM"))
        ps_sc = attn_ctx.enter_context(tc.tile_pool(name="ps_sc", bufs=2, space="PSUM"))
        ps_av = attn_ctx.enter_context(tc.tile_pool(name="ps_av", bufs=3, space="PSUM"))

        def emit_qkT(t):
            # qkT for pair t: M-tiles t (q rows) and 8+t (k rows)
            pair_tiles = []
            for idx, which in enumerate((t, NPAIR + t)):
                dst = p_qk.tile([P, S], F32R, name=("qpair" if idx == 0 else "kpair"))
                wq = [p_wqk.tile([P, P], F32R, name="wqk") for _ in range(NT)]
                for kt in range(NT):
                    nc.sync.dma_start(
                        wq[kt][:],
                        wattn_d[kt * P : (kt + 1) * P, which * P : (which + 1) * P],
                    )
                for nn in range(NQC):
                    ps = ps_qk.tile([P, QW], F32, name="qkps")
                    for kt in range(NT):
                        nc.tensor.matmul(
                            ps[:], wq[kt][:], xT[kt][:, nn * QW : (nn + 1) * QW],
                            start=(kt == 0), stop=False,
                        )
                    nc.tensor.matmul(
                        ps[:], battn_sb[:, which * P : (which + 1) * P],
                        ones_row[:],
                        start=False, stop=True,
                    )
                    nc.vector.tensor_copy(dst[:, nn * QW : (nn + 1) * QW], ps[:])
                pair_tiles.append(dst)
            return pair_tiles

        def emit_scores_exp(t, q_pair, k_pair):
            for qc in range(NQC):
                live = range(0, 4 if qc == 0 else NT)
                for ki in live:
                    has_diag = (qc * QW) <= ki * P < (qc + 1) * QW
                    offd = ki * P - qc * QW
                    off0 = max(0, offd)
                    sts = []
                    for h, (hb, tp) in enumerate(((0, (0, 0)), (64, (64, 0)))):
                        stt = ps_sc.tile([P, QW], F32, name=("sta" if h == 0 else "stb"))
                        nc.tensor.matmul(
                            stt[:],
                            k_pair[hb : hb + 64, ki * P : (ki + 1) * P],
                            q_pair[hb : hb + 64, qc * QW : (qc + 1) * QW],
                            start=True, stop=not has_diag,
                            tile_position=tp,
                        )
                        sts.append(stt)
                    if has_diag:
                        for stt in sts:
                            nc.tensor.matmul(
                                stt[:, offd : offd + P], identb[:], tpat[:],
                                start=False, stop=True,
                            )
                    for stt, wTh in ((sts[0], wT_A), (sts[1], wT_B)):
                        nc.scalar.activation(
                            wTh[ki][:, qc * QW + off0 : (qc + 1) * QW],
                            stt[:, off0:QW],
                            EXP, scale=SCALE,
                        )

        def emit_attnv(t):
            for qc in range(NQC):
                live = range(0, 4 if qc == 0 else NT)
                nlive = 4 if qc == 0 else NT
                for h, wTh in ((0, wT_A), (1, wT_B)):
                    av = ps_av.tile([D + 1, QW], F32, name="av")
                    for ki in live:
                        nc.tensor.matmul(
                            av[:, :], v[ki][:, 2 * t + h, :],
                            wTh[ki][:, qc * QW : (qc + 1) * QW],
                            start=(ki == 0), stop=(ki == nlive - 1),
                        )
                    rec = p_misc.tile([1, QW], F32, name="rec")
                    nc.vector.reciprocal(rec[0:1, :], av[D : D + 1, :])
                    bcast = p_bc.tile([D, QW], F32, name="bcast")
                    nc.gpsimd.partition_broadcast(bcast[:, :], rec[0:1, :], channels=D)
                    nc.vector.tensor_mul(
                        aT[t][h * D : (h + 1) * D, qc * QW : (qc + 1) * QW],
                        av[0:D, :], bcast[:],
                    )

        # software pipeline: qkT(t+1) is emitted between scores/exp(t) and
        # attn x V(t), so the PE chews on qkT matmuls while ScalarE drains exps
        cur = emit_qkT(0)
        for t in range(NPAIR):
            emit_scores_exp(t, cur[0], cur[1])
            nxt = emit_qkT(t + 1) if t + 1 < NPAIR else None
            emit_attnv(t)
            cur = nxt

        attn_ctx.close()

        # ---------- phase 5: out = a @ w_proj + b ----------
        with tc.tile_pool(name="p_wp", bufs=1) as p_wp, \
             tc.tile_pool(name="p_out", bufs=2) as p_out, \
             tc.tile_pool(name="ps_o", bufs=2, space="PSUM") as ps_o:
            bproj_sb = p_wp.tile([1, NX], F32R, name="bproj")
            nc.sync.dma_start(bproj_sb[:], bproj_d.rearrange("(a b) -> a b", a=1))
            wp = [p_wp.tile([P, NX], F32R, name=f"wp{i}") for i in range(NT)]
            for kt in range(NT):
                nc.sync.dma_start(wp[kt][:], wproj_d[kt * P : (kt + 1) * P, :])
            for mt in range(NT):
                ot = p_out.tile([P, NX], F32, name="ot")
                for nn in range(NQC):
                    ps = ps_o.tile([P, QW], F32, name="ops")
                    for kt in range(NT):
                        nc.tensor.matmul(
                            ps[:], aT[kt][:, mt * P : (mt + 1) * P],
                            wp[kt][:, nn * QW : (nn + 1) * QW],
                            start=(kt == 0), stop=False,
                        )
                    nc.tensor.matmul(
                        ps[:], ones_row[:, 0:P],
                        bproj_sb[:, nn * QW : (nn + 1) * QW],
                        start=False, stop=True,
                    )
                    nc.vector.tensor_copy(ot[:, nn * QW : (nn + 1) * QW], ps[:])
                nc.sync.dma_start(out_d[mt * P : (mt + 1) * P, :], ot[:])

    nc.compile()
    return nc


_NC_CACHE = {}


def _get_nc():
    if "nc" not in _NC_CACHE:
        _NC_CACHE["nc"] = build_nc()
    return _NC_CACHE["nc"]


def _cst_array():
    cst = np.zeros((P, 2 * NX), dtype=np.float32)
    cst[:, :NX] = 1.0
    return cst


def kernel(x, w_attn, b_attn, w_proj, b_proj):
    x = np.asarray(x, dtype=np.float32)
    w_attn = np.asarray(w_attn, dtype=np.float32)
    b_attn = np.asarray(b_attn, dtype=np.float32)
    w_proj = np.asarray(w_proj, dtype=np.float32)
    b_proj = np.asarray(b_proj, dtype=np.float32)

    nc = _get_nc()
    cst = _cst_array()
    B = x.shape[0]
    in_maps = [
        {
            "x": x[b],
            "w_attn": w_attn,
            "b_attn": b_attn,
            "w_proj": w_proj,
            "b_proj": b_proj,
            "cst": cst,
        }
        for b in range(B)
    ]
    res = run_bass_kernel_spmd(nc, in_maps, list(range(B)))
    return np.stack([res.results[b]["out"] for b in range(B)], axis=0).astype(np.float32)


# revision 15
# speedup vs baseline: 1.0600x; 1.0600x over previous
"""Causal multi-head attention block (qkv -> causal softmax(qk^T/sqrt(d)) v -> proj)
for Trainium2, batch-sharded across 8 NeuronCores (1 batch element per core).

Self-contained: hardcodes shapes B=8, S=1024, NX=1024, H=16, D=64.

Per-core dataflow (all matmuls fp32r = full-rate reduced-precision fp32):
  x [S,NX] --PE transpose--> xT [NX,S]
  v  = xT.T @ w_v   (+b)    -> v  [S,NX]   (natural)
  qkT = w_qk.T...: matmul(lhsT=w_attn[:, :2048], rhs=xT) -> qT,kT [2048,S] (transposed)
  per head pair (2 heads/128 partitions):
    sT[k,q] = matmul(lhsT=kT_h[64,k-chunk], rhs=qT_h[64,q-chunk])   row-tiled x2 heads
    causal:   fully-masked tiles skipped; diagonal 128x128 blocks get
              += I.T @ T  (T = -30000 below diagonal) accumulated in PSUM
    wT = Exp(sT/8)                       (ScalarE, PSUM->SBUF fp32r)
    aT_u = matmul(lhsT=v_h[k,64], rhs=wT) col-tiled x2 heads -> [128(d),q]
    sum  = matmul(lhsT=ones[128,1], rhs=wT) col-tiled x2 heads (M=1 each)
    aT   = aT_u * broadcast(1/sum)       (VectorE eviction, fp32r)
  out = matmul(lhsT=aT, rhs=w_proj) (+b) -> [S,NX]
"""

import sys

if "/opt/trn_rl_repo" not in sys.path:
    sys.path.insert(0, "/opt/trn_rl_repo")

import numpy as np
from contextlib import ExitStack

import concourse.bass as bass
import concourse.mybir as mybir
import concourse.tile as tile
from concourse import bacc
from concourse.bass_utils import run_bass_kernel_spmd
from concourse.masks import make_identity

F32 = mybir.dt.float32
F32R = mybir.dt.float32r
BF16 = mybir.dt.bfloat16
EXP = mybir.ActivationFunctionType.Exp

P = 128
S = 1024
NX = 1024
H = 16
D = 64
NT = S // P            # 8 partition tiles along S or NX
NQC = 2                # 512-wide q windows
QW = 512
NPAIR = H // 2         # 8 head pairs
SCALE = 0.125          # 1/sqrt(64)
NEGBIG = -30000.0      # pre-scale mask add; exp(SCALE*NEGBIG) == 0 in fp32


def build_nc(with_bias=False):
    nc = bacc.Bacc("TRN2", target_bir_lowering=False, debug=False)

    x_d = nc.dram_tensor("x", [S, NX], F32R, kind="ExternalInput").ap()
    wattn_d = nc.dram_tensor("w_attn", [NX, 3 * NX], F32R, kind="ExternalInput").ap()
    battn_d = nc.dram_tensor("b_attn", [3 * NX], F32R, kind="ExternalInput").ap()
    wproj_d = nc.dram_tensor("w_proj", [NX, NX], F32R, kind="ExternalInput").ap()
    bproj_d = nc.dram_tensor("b_proj", [NX], F32R, kind="ExternalInput").ap()
    cst_d = nc.dram_tensor("cst", [P, 2 * NX], F32R, kind="ExternalInput").ap()
    out_d = nc.dram_tensor("out", [S, NX], F32, kind="ExternalOutput").ap()

    with tile.TileContext(nc) as tc, ExitStack() as ctx:
        const = ctx.enter_context(tc.tile_pool(name="const", bufs=1))
        p_xT = ctx.enter_context(tc.tile_pool(name="p_xT", bufs=1))
        p_v = ctx.enter_context(tc.tile_pool(name="p_v", bufs=1))
        p_aT = ctx.enter_context(tc.tile_pool(name="p_aT", bufs=1))

        # ---------- constants ----------
        ident32 = const.tile([P, P], F32R)
        nc.sync.dma_start(ident32[:], cst_d[:, QW : QW + P])
        identb = const.tile([P, P], BF16)
        make_identity(nc, identb)
        tpat = const.tile([P, P], BF16)
        nc.gpsimd.memset(tpat[:], 0.0)
        # tpat[r, c] = 0 if r <= c else NEGBIG   (k on partitions, q on free)
        nc.gpsimd.affine_select(
            out=tpat[:], in_=tpat[:],
            compare_op=mybir.AluOpType.is_ge,
            fill=NEGBIG, base=0,
            pattern=[[1, P]], channel_multiplier=-1,
        )
        ones_row = const.tile([1, QW], F32R)
        nc.sync.dma_start(ones_row[:], cst_d[0:1, 0:QW])
        battn_sb = const.tile([1, 2 * NX], F32R)
        nc.sync.dma_start(battn_sb[:], battn_d.rearrange("(a b) -> a b", a=1)[:, 0 : 2 * NX])

        xT = [p_xT.tile([P, S], F32R, name=f"xT{i}") for i in range(NT)]

        # ---------- phase 1: x transpose ----------
        with tc.tile_pool(name="p_x", bufs=2) as p_x, \
             tc.tile_pool(name="ps_tr", bufs=2, space="PSUM") as ps_tr:
            for st in range(NT):
                xt_in = p_x.tile([P, NX], F32R, name="xin")
                nc.sync.dma_start(xt_in[:], x_d[st * P : (st + 1) * P, :])
                for nt in range(NT):
                    pst = ps_tr.tile([P, P], F32R, name="trp")
                    nc.tensor.transpose(pst[:], xt_in[:, nt * P : (nt + 1) * P], ident32[:])
                    nc.vector.tensor_copy(xT[nt][:, st * P : (st + 1) * P], pst[:])

        # ---------- phase 2: v = x @ w_v + b_v (natural [S, NX]), augmented with a
        # ones column per head: v_aug[s, h, 0:64] = v head h, v_aug[s, h, 64] = 1
        v = [p_v.tile([P, H, D + 1], F32R, name=f"v{i}") for i in range(NT)]
        with tc.tile_pool(name="p_wv", bufs=1) as p_wv, \
             tc.tile_pool(name="ps_v", bufs=2, space="PSUM") as ps_v:
            battn_v = p_wv.tile([1, NX], F32R, name="battn_v")
            nc.sync.dma_start(
                battn_v[:], battn_d.rearrange("(a b) -> a b", a=1)[:, 2 * NX : 3 * NX]
            )
            wv = [p_wv.tile([P, NX], F32R, name=f"wv{i}") for i in range(NT)]
            for kt in range(NT):
                nc.sync.dma_start(wv[kt][:], wattn_d[kt * P : (kt + 1) * P, 2 * NX : 3 * NX])
            for mt in range(NT):
                nc.sync.dma_start(v[mt][:, :, D : D + 1], cst_d[:, 0:H])
            for mt in range(NT):
                for nn in range(NQC):
                    ps = ps_v.tile([P, QW], F32, name="vps")
                    for kt in range(NT):
                        nc.tensor.matmul(
                            ps[:], xT[kt][:, mt * P : (mt + 1) * P],
                            wv[kt][:, nn * QW : (nn + 1) * QW],
                            start=(kt == 0), stop=(kt == NT - 1 and not with_bias),
                        )
                    if with_bias:
                        nc.tensor.matmul(
                            ps[:], ones_row[:, 0:P],
                            battn_v[:, nn * QW : (nn + 1) * QW],
                            start=False, stop=True,
                        )
                    nc.vector.tensor_copy(
                        v[mt][:, nn * (QW // D) : (nn + 1) * (QW // D), 0:D],
                        ps[:].rearrange("p (h d) -> p h d", d=D),
                    )

        # ---------- phases 3+4: per-pair qkT then attention ----------
        aT = [p_aT.tile([P, S], F32R, name=f"aT{i}") for i in range(NT)]

        attn_ctx = ExitStack()
        p_qk = attn_ctx.enter_context(tc.tile_pool(name="p_qk", bufs=2))
        p_wT = attn_ctx.enter_context(tc.tile_pool(name="p_wT", bufs=1))
        p_wqk = attn_ctx.enter_context(tc.tile_pool(name="p_wqk", bufs=16))
        p_misc = attn_ctx.enter_context(tc.tile_pool(name="p_misc", bufs=3))
        p_bc = attn_ctx.enter_context(tc.tile_pool(name="p_bc", bufs=3))
        wT_A = [p_wT.tile([P, S], F32R, name=f"wTA{i}") for i in range(NT)]
        wT_B = [p_wT.tile([P, S], F32R, name=f"wTB{i}") for i in range(NT)]
        # dead (fully-masked) column ranges zeroed once; exp evictions never touch them
        for ki in range(1, NT):
            nc.sync.dma_start(wT_A[ki][:, 0 : ki * P], cst_d[:, NX : NX + ki * P])
            nc.sync.dma_start(wT_B[ki][:, 0 : ki * P], cst_d[:, NX : NX + ki * P])

        ps_qk = attn_ctx.enter_context(tc.tile_pool(name="ps_qk", bufs=1, space="PSUM"))
        ps_sc = attn_ctx.enter_context(tc.tile_pool(name="ps_sc", bufs=2, space="PSUM"))
        ps_av = attn_ctx.enter_context(tc.tile_pool(name="ps_av", bufs=3, space="PSUM"))

        def emit_qkT(t):
            # qkT for pair t: M-tiles t (q rows) and 8+t (k rows)
            pair_tiles = []
            for idx, which in enumerate((t, NPAIR + t)):
                dst = p_qk.tile([P, S], F32R, name=("qpair" if idx == 0 else "kpair"))
                wq = [p_wqk.tile([P, P], F32R, name="wqk") for _ in range(NT)]
                for kt in range(NT):
                    nc.sync.dma_start(
                        wq[kt][:],
                        wattn_d[kt * P : (kt + 1) * P, which * P : (which + 1) * P],
                    )
                for nn in range(NQC):
                    ps = ps_qk.tile([P, QW], F32, name="qkps")
                    for kt in range(NT):
                        nc.tensor.matmul(
                            ps[:], wq[kt][:], xT[kt][:, nn * QW : (nn + 1) * QW],
                            start=(kt == 0), stop=(kt == NT - 1 and not with_bias),
                        )
                    if with_bias:
                        nc.tensor.matmul(
                            ps[:], battn_sb[:, which * P : (which + 1) * P],
                            ones_row[:],
                            start=False, stop=True,
                        )
                    nc.vector.tensor_copy(dst[:, nn * QW : (nn + 1) * QW], ps[:])
                pair_tiles.append(dst)
            return pair_tiles

        def emit_scores_exp(t, q_pair, k_pair):
            for qc in range(NQC):
                live = range(0, 4 if qc == 0 else NT)
                for ki in live:
                    has_diag = (qc * QW) <= ki * P < (qc + 1) * QW
                    offd = ki * P - qc * QW
                    off0 = max(0, offd)
                    sts = []
                    for h, (hb, tp) in enumerate(((0, (0, 0)), (64, (64, 0)))):
                        stt = ps_sc.tile([P, QW], F32, name=("sta" if h == 0 else "stb"))
                        nc.tensor.matmul(
                            stt[:],
                            k_pair[hb : hb + 64, ki * P : (ki + 1) * P],
                            q_pair[hb : hb + 64, qc * QW : (qc + 1) * QW],
                            start=True, stop=not has_diag,
                            tile_position=tp,
                        )
                        sts.append(stt)
                    if has_diag:
                        for stt in sts:
                            nc.tensor.matmul(
                                stt[:, offd : offd + P], identb[:], tpat[:],
                                start=False, stop=True,
                            )
                    for stt, wTh in ((sts[0], wT_A), (sts[1], wT_B)):
                        nc.scalar.activation(
                            wTh[ki][:, qc * QW + off0 : (qc + 1) * QW],
                            stt[:, off0:QW],
                            EXP, scale=SCALE,
                        )

        def emit_attnv(t):
            for qc in range(NQC):
                live = range(0, 4 if qc == 0 else NT)
                nlive = 4 if qc == 0 else NT
                for h, wTh in ((0, wT_A), (1, wT_B)):
                    av = ps_av.tile([D + 1, QW], F32, name="av")
                    for ki in live:
                        nc.tensor.matmul(
                            av[:, :], v[ki][:, 2 * t + h, :],
                            wTh[ki][:, qc * QW : (qc + 1) * QW],
                            start=(ki == 0), stop=(ki == nlive - 1),
                        )
                    rec = p_misc.tile([1, QW], F32, name="rec")
                    nc.vector.reciprocal(rec[0:1, :], av[D : D + 1, :])
                    bcast = p_bc.tile([D, QW], F32, name="bcast")
                    nc.gpsimd.partition_broadcast(bcast[:, :], rec[0:1, :], channels=D)
                    nc.vector.tensor_mul(
                        aT[t][h * D : (h + 1) * D, qc * QW : (qc + 1) * QW],
                        av[0:D, :], bcast[:],
                    )

        # software pipeline: qkT(t+1) is emitted between scores/exp(t) and
        # attn x V(t), so the PE chews on qkT matmuls while ScalarE drains exps
        cur = emit_qkT(0)
        for t in range(NPAIR):
            emit_scores_exp(t, cur[0], cur[1])
            nxt = emit_qkT(t + 1) if t + 1 < NPAIR else None
            emit_attnv(t)
            cur = nxt

        attn_ctx.close()

        # ---------- phase 5: out = a @ w_proj + b ----------
        with tc.tile_pool(name="p_wp", bufs=1) as p_wp, \
             tc.tile_pool(name="p_out", bufs=2) as p_out, \
             tc.tile_pool(name="ps_o", bufs=2, space="PSUM") as ps_o:
            bproj_sb = p_wp.tile([1, NX], F32R, name="bproj")
            nc.sync.dma_start(bproj_sb[:], bproj_d.rearrange("(a b) -> a b", a=1))
            wp = [p_wp.tile([P, NX], F32R, name=f"wp{i}") for i in range(NT)]
            for kt in range(NT):
                nc.sync.dma_start(wp[kt][:], wproj_d[kt * P : (kt + 1) * P, :])
            for mt in range(NT):
                ot = p_out.tile([P, NX], F32, name="ot")
                for nn in range(NQC):
                    ps = ps_o.tile([P, QW], F32, name="ops")
                    for kt in range(NT):
                        nc.tensor.matmul(
                            ps[:], aT[kt][:, mt * P : (mt + 1) * P],
                            wp[kt][:, nn * QW : (nn + 1) * QW],
                            start=(kt == 0), stop=(kt == NT - 1 and not with_bias),
                        )
                    if with_bias:
                        nc.tensor.matmul(
                            ps[:], ones_row[:, 0:P],
                            bproj_sb[:, nn * QW : (nn + 1) * QW],
                            start=False, stop=True,
                        )
                    nc.vector.tensor_copy(ot[:, nn * QW : (nn + 1) * QW], ps[:])
                nc.sync.dma_start(out_d[mt * P : (mt + 1) * P, :], ot[:])

    nc.compile()
    return nc


_NC_CACHE = {}


def _get_nc(with_bias=False):
    key = "nc_bias" if with_bias else "nc"
    if key not in _NC_CACHE:
        _NC_CACHE[key] = build_nc(with_bias)
    return _NC_CACHE[key]


def _cst_array():
    cst = np.zeros((P, 2 * NX), dtype=np.float32)
    cst[:, :QW] = 1.0
    cst[:, QW : QW + P] = np.eye(P, dtype=np.float32)
    return cst


def kernel(x, w_attn, b_attn, w_proj, b_proj):
    x = np.asarray(x, dtype=np.float32)
    w_attn = np.asarray(w_attn, dtype=np.float32)
    b_attn = np.asarray(b_attn, dtype=np.float32)
    w_proj = np.asarray(w_proj, dtype=np.float32)
    b_proj = np.asarray(b_proj, dtype=np.float32)

    with_bias = bool(np.any(b_attn) or np.any(b_proj))
    nc = _get_nc(with_bias)
    cst = _cst_array()
    B = x.shape[0]
    in_maps = [
        {
            "x": x[b],
            "w_attn": w_attn,
            "b_attn": b_attn,
            "w_proj": w_proj,
            "b_proj": b_proj,
            "cst": cst,
        }
        for b in range(B)
    ]
    res = run_bass_kernel_spmd(nc, in_maps, list(range(B)))
    return np.stack([res.results[b]["out"] for b in range(B)], axis=0).astype(np.float32)


# revision 17
# speedup vs baseline: 1.0729x; 1.0122x over previous
"""Causal multi-head attention block (qkv -> causal softmax(qk^T/sqrt(d)) v -> proj)
for Trainium2, batch-sharded across 8 NeuronCores (1 batch element per core).

Self-contained: hardcodes shapes B=8, S=1024, NX=1024, H=16, D=64.

Per-core dataflow (all matmuls fp32r = full-rate reduced-precision fp32):
  x [S,NX] --PE transpose--> xT [NX,S]
  v  = xT.T @ w_v   (+b)    -> v  [S,NX]   (natural)
  qkT = w_qk.T...: matmul(lhsT=w_attn[:, :2048], rhs=xT) -> qT,kT [2048,S] (transposed)
  per head pair (2 heads/128 partitions):
    sT[k,q] = matmul(lhsT=kT_h[64,k-chunk], rhs=qT_h[64,q-chunk])   row-tiled x2 heads
    causal:   fully-masked tiles skipped; diagonal 128x128 blocks get
              += I.T @ T  (T = -30000 below diagonal) accumulated in PSUM
    wT = Exp(sT/8)                       (ScalarE, PSUM->SBUF fp32r)
    aT_u = matmul(lhsT=v_h[k,64], rhs=wT) col-tiled x2 heads -> [128(d),q]
    sum  = matmul(lhsT=ones[128,1], rhs=wT) col-tiled x2 heads (M=1 each)
    aT   = aT_u * broadcast(1/sum)       (VectorE eviction, fp32r)
  out = matmul(lhsT=aT, rhs=w_proj) (+b) -> [S,NX]
"""

import sys

if "/opt/trn_rl_repo" not in sys.path:
    sys.path.insert(0, "/opt/trn_rl_repo")

import numpy as np
from contextlib import ExitStack

import concourse.bass as bass
import concourse.mybir as mybir
import concourse.tile as tile
from concourse import bacc
from concourse.bass_utils import run_bass_kernel_spmd
from concourse.masks import make_identity

F32 = mybir.dt.float32
F32R = mybir.dt.float32r
BF16 = mybir.dt.bfloat16
EXP = mybir.ActivationFunctionType.Exp

P = 128
S = 1024
NX = 1024
H = 16
D = 64
NT = S // P            # 8 partition tiles along S or NX
NQC = 2                # 512-wide q windows
QW = 512
NPAIR = H // 2         # 8 head pairs
SCALE = 0.125          # 1/sqrt(64)
NEGBIG = -30000.0      # pre-scale mask add; exp(SCALE*NEGBIG) == 0 in fp32


def build_nc(with_bias=False):
    nc = bacc.Bacc("TRN2", target_bir_lowering=False, debug=False)

    x_d = nc.dram_tensor("x", [S, NX], F32R, kind="ExternalInput").ap()
    wattn_d = nc.dram_tensor("w_attn", [NX, 3 * NX], F32R, kind="ExternalInput").ap()
    battn_d = nc.dram_tensor("b_attn", [3 * NX], F32R, kind="ExternalInput").ap()
    wproj_d = nc.dram_tensor("w_proj", [NX, NX], F32R, kind="ExternalInput").ap()
    bproj_d = nc.dram_tensor("b_proj", [NX], F32R, kind="ExternalInput").ap()
    cst_d = nc.dram_tensor("cst", [P, 2 * NX], F32R, kind="ExternalInput").ap()
    out_d = nc.dram_tensor("out", [S, NX], F32, kind="ExternalOutput").ap()

    with tile.TileContext(nc) as tc, ExitStack() as ctx:
        const = ctx.enter_context(tc.tile_pool(name="const", bufs=1))
        p_xT = ctx.enter_context(tc.tile_pool(name="p_xT", bufs=1))
        p_v = ctx.enter_context(tc.tile_pool(name="p_v", bufs=1))
        p_aT = ctx.enter_context(tc.tile_pool(name="p_aT", bufs=1))

        # ---------- constants ----------
        ident32 = const.tile([P, P], F32R)
        nc.sync.dma_start(ident32[:], cst_d[:, QW : QW + P])
        identb = const.tile([P, P], BF16)
        make_identity(nc, identb)
        tpat = const.tile([P, P], BF16)
        nc.gpsimd.memset(tpat[:], 0.0)
        # tpat[r, c] = 0 if r <= c else NEGBIG   (k on partitions, q on free)
        nc.gpsimd.affine_select(
            out=tpat[:], in_=tpat[:],
            compare_op=mybir.AluOpType.is_ge,
            fill=NEGBIG, base=0,
            pattern=[[1, P]], channel_multiplier=-1,
        )
        ones_row = const.tile([1, QW], F32R)
        nc.sync.dma_start(ones_row[:], cst_d[0:1, 0:QW])
        battn_sb = const.tile([1, 2 * NX], F32R)
        nc.sync.dma_start(battn_sb[:], battn_d.rearrange("(a b) -> a b", a=1)[:, 0 : 2 * NX])

        xT = [p_xT.tile([P, S], F32R, name=f"xT{i}") for i in range(NT)]

        # ---------- phase 1: x transpose ----------
        with tc.tile_pool(name="p_x", bufs=2) as p_x, \
             tc.tile_pool(name="ps_tr", bufs=2, space="PSUM") as ps_tr:
            for st2 in range(NT // 2):
                xt_in = p_x.tile([P, 2, NX], F32R, name="xin")
                nc.sync.dma_start(
                    xt_in[:],
                    x_d[st2 * 2 * P : (st2 + 1) * 2 * P, :].rearrange(
                        "(a p) f -> p a f", p=P
                    ),
                )
                for a in range(2):
                    st = 2 * st2 + a
                    for nt in range(NT):
                        pst = ps_tr.tile([P, P], F32R, name="trp")
                        nc.tensor.transpose(
                            pst[:], xt_in[:, a, nt * P : (nt + 1) * P], ident32[:]
                        )
                        nc.vector.tensor_copy(xT[nt][:, st * P : (st + 1) * P], pst[:])

        # ---------- phase 2: v = x @ w_v + b_v (natural [S, NX]), augmented with a
        # ones column per head: v_aug[s, h, 0:64] = v head h, v_aug[s, h, 64] = 1
        v = [p_v.tile([P, H, D + 1], F32R, name=f"v{i}") for i in range(NT)]
        with tc.tile_pool(name="p_wv", bufs=1) as p_wv, \
             tc.tile_pool(name="ps_v", bufs=2, space="PSUM") as ps_v:
            battn_v = p_wv.tile([1, NX], F32R, name="battn_v")
            nc.sync.dma_start(
                battn_v[:], battn_d.rearrange("(a b) -> a b", a=1)[:, 2 * NX : 3 * NX]
            )
            wv = [p_wv.tile([P, NX], F32R, name=f"wv{i}") for i in range(NT)]
            for kt in range(NT):
                nc.sync.dma_start(wv[kt][:], wattn_d[kt * P : (kt + 1) * P, 2 * NX : 3 * NX])
            for mt in range(NT):
                nc.sync.dma_start(v[mt][:, :, D : D + 1], cst_d[:, 0:H])
            for mt in range(NT):
                for nn in range(NQC):
                    ps = ps_v.tile([P, QW], F32, name="vps")
                    for kt in range(NT):
                        nc.tensor.matmul(
                            ps[:], xT[kt][:, mt * P : (mt + 1) * P],
                            wv[kt][:, nn * QW : (nn + 1) * QW],
                            start=(kt == 0), stop=(kt == NT - 1 and not with_bias),
                        )
                    if with_bias:
                        nc.tensor.matmul(
                            ps[:], ones_row[:, 0:P],
                            battn_v[:, nn * QW : (nn + 1) * QW],
                            start=False, stop=True,
                        )
                    nc.vector.tensor_copy(
                        v[mt][:, nn * (QW // D) : (nn + 1) * (QW // D), 0:D],
                        ps[:].rearrange("p (h d) -> p h d", d=D),
                    )

        # ---------- phases 3+4: per-pair qkT then attention ----------
        aT = [p_aT.tile([P, S], F32R, name=f"aT{i}") for i in range(NT)]

        attn_ctx = ExitStack()
        p_qk = attn_ctx.enter_context(tc.tile_pool(name="p_qk", bufs=2))
        p_wT = attn_ctx.enter_context(tc.tile_pool(name="p_wT", bufs=1))
        p_wqk = attn_ctx.enter_context(tc.tile_pool(name="p_wqk", bufs=3))
        p_misc = attn_ctx.enter_context(tc.tile_pool(name="p_misc", bufs=2))
        p_bc = attn_ctx.enter_context(tc.tile_pool(name="p_bc", bufs=2))
        wT_A = [p_wT.tile([P, S], F32R, name=f"wTA{i}") for i in range(NT)]
        wT_B = [p_wT.tile([P, S], F32R, name=f"wTB{i}") for i in range(NT)]
        # dead (fully-masked) column ranges zeroed once; exp evictions never touch them
        for ki in range(1, NT):
            nc.sync.dma_start(wT_A[ki][:, 0 : ki * P], cst_d[:, NX : NX + ki * P])
            nc.sync.dma_start(wT_B[ki][:, 0 : ki * P], cst_d[:, NX : NX + ki * P])

        ps_qk = attn_ctx.enter_context(tc.tile_pool(name="ps_qk", bufs=1, space="PSUM"))
        ps_sc = attn_ctx.enter_context(tc.tile_pool(name="ps_sc", bufs=2, space="PSUM"))
        ps_av = attn_ctx.enter_context(tc.tile_pool(name="ps_av", bufs=3, space="PSUM"))

        def emit_qkT(t):
            # qkT for pair t: M-tiles t (q rows) and 8+t (k rows)
            pair_tiles = []
            for idx, which in enumerate((t, NPAIR + t)):
                dst = p_qk.tile([P, S], F32R, name=("qpair" if idx == 0 else "kpair"))
                wq = p_wqk.tile([P, NT, P], F32R, name="wqk")
                nc.sync.dma_start(
                    wq[:],
                    wattn_d[:, which * P : (which + 1) * P].rearrange(
                        "(kt p) c -> p kt c", p=P
                    ),
                )
                for nn in range(NQC):
                    ps = ps_qk.tile([P, QW], F32, name="qkps")
                    for kt in range(NT):
                        nc.tensor.matmul(
                            ps[:], wq[:, kt, :], xT[kt][:, nn * QW : (nn + 1) * QW],
                            start=(kt == 0), stop=(kt == NT - 1 and not with_bias),
                        )
                    if with_bias:
                        nc.tensor.matmul(
                            ps[:], battn_sb[:, which * P : (which + 1) * P],
                            ones_row[:],
                            start=False, stop=True,
                        )
                    nc.vector.tensor_copy(dst[:, nn * QW : (nn + 1) * QW], ps[:])
                pair_tiles.append(dst)
            return pair_tiles

        def emit_scores_exp(t, q_pair, k_pair):
            for qc in range(NQC):
                live = range(0, 4 if qc == 0 else NT)
                for ki in live:
                    has_diag = (qc * QW) <= ki * P < (qc + 1) * QW
                    offd = ki * P - qc * QW
                    off0 = max(0, offd)
                    nmm = max(256, QW - off0)   # fp32r needs N>=256 for full rate
                    offm = QW - nmm
                    sts = []
                    for h, (hb, tp) in enumerate(((0, (0, 0)), (64, (64, 0)))):
                        stt = ps_sc.tile([P, QW], F32, name=("sta" if h == 0 else "stb"))
                        nc.tensor.matmul(
                            stt[:, offm:QW],
                            k_pair[hb : hb + 64, ki * P : (ki + 1) * P],
                            q_pair[hb : hb + 64, qc * QW + offm : (qc + 1) * QW],
                            start=True, stop=not has_diag,
                            tile_position=tp,
                        )
                        sts.append(stt)
                    if has_diag:
                        for stt in sts:
                            nc.tensor.matmul(
                                stt[:, offd : offd + P], identb[:], tpat[:],
                                start=False, stop=True,
                            )
                    for stt, wTh in ((sts[0], wT_A), (sts[1], wT_B)):
                        nc.scalar.activation(
                            wTh[ki][:, qc * QW + off0 : (qc + 1) * QW],
                            stt[:, off0:QW],
                            EXP, scale=SCALE,
                        )

        def emit_attnv(t):
            for qc in range(NQC):
                live = range(0, 4 if qc == 0 else NT)
                nlive = 4 if qc == 0 else NT
                for h, wTh in ((0, wT_A), (1, wT_B)):
                    av = ps_av.tile([D + 1, QW], F32, name="av")
                    for ki in live:
                        off0 = max(0, ki * P - qc * QW)
                        offm = QW - max(256, QW - off0)
                        nc.tensor.matmul(
                            av[:, offm:QW], v[ki][:, 2 * t + h, :],
                            wTh[ki][:, qc * QW + offm : (qc + 1) * QW],
                            start=(ki == 0), stop=(ki == nlive - 1),
                        )
                    rec = p_misc.tile([1, QW], F32, name="rec")
                    nc.vector.reciprocal(rec[0:1, :], av[D : D + 1, :])
                    bcast = p_bc.tile([D, QW], F32, name="bcast")
                    nc.gpsimd.partition_broadcast(bcast[:, :], rec[0:1, :], channels=D)
                    nc.vector.tensor_mul(
                        aT[t][h * D : (h + 1) * D, qc * QW : (qc + 1) * QW],
                        av[0:D, :], bcast[:],
                    )

        # software pipeline: qkT(t+1) is emitted between scores/exp(t) and
        # attn x V(t), so the PE chews on qkT matmuls while ScalarE drains exps
        cur = emit_qkT(0)
        for t in range(NPAIR):
            emit_scores_exp(t, cur[0], cur[1])
            nxt = emit_qkT(t + 1) if t + 1 < NPAIR else None
            emit_attnv(t)
            cur = nxt

        attn_ctx.close()

        # ---------- phase 5: out = a @ w_proj + b ----------
        with tc.tile_pool(name="p_wp", bufs=1) as p_wp, \
             tc.tile_pool(name="p_out", bufs=2) as p_out, \
             tc.tile_pool(name="ps_o", bufs=2, space="PSUM") as ps_o:
            bproj_sb = p_wp.tile([1, NX], F32R, name="bproj")
            nc.sync.dma_start(bproj_sb[:], bproj_d.rearrange("(a b) -> a b", a=1))
            wp = [p_wp.tile([P, NX], F32R, name=f"wp{i}") for i in range(NT)]
            for kt in range(NT):
                nc.sync.dma_start(wp[kt][:], wproj_d[kt * P : (kt + 1) * P, :])
            for mt in range(NT):
                ot = p_out.tile([P, NX], F32, name="ot")
                for nn in range(NQC):
                    ps = ps_o.tile([P, QW], F32, name="ops")
                    for kt in range(NT):
                        nc.tensor.matmul(
                            ps[:], aT[kt][:, mt * P : (mt + 1) * P],
                            wp[kt][:, nn * QW : (nn + 1) * QW],
                            start=(kt == 0), stop=(kt == NT - 1 and not with_bias),
                        )
                    if with_bias:
                        nc.tensor.matmul(
                            ps[:], ones_row[:, 0:P],
                            bproj_sb[:, nn * QW : (nn + 1) * QW],
                            start=False, stop=True,
                        )
                    nc.vector.tensor_copy(ot[:, nn * QW : (nn + 1) * QW], ps[:])
                nc.sync.dma_start(out_d[mt * P : (mt + 1) * P, :], ot[:])

    nc.compile()
    return nc


_NC_CACHE = {}


def _get_nc(with_bias=False):
    key = "nc_bias" if with_bias else "nc"
    if key not in _NC_CACHE:
        _NC_CACHE[key] = build_nc(with_bias)
    return _NC_CACHE[key]


def _cst_array():
    cst = np.zeros((P, 2 * NX), dtype=np.float32)
    cst[:, :QW] = 1.0
    cst[:, QW : QW + P] = np.eye(P, dtype=np.float32)
    return cst


def kernel(x, w_attn, b_attn, w_proj, b_proj):
    x = np.asarray(x, dtype=np.float32)
    w_attn = np.asarray(w_attn, dtype=np.float32)
    b_attn = np.asarray(b_attn, dtype=np.float32)
    w_proj = np.asarray(w_proj, dtype=np.float32)
    b_proj = np.asarray(b_proj, dtype=np.float32)

    with_bias = bool(np.any(b_attn) or np.any(b_proj))
    nc = _get_nc(with_bias)
    cst = _cst_array()
    B = x.shape[0]
    in_maps = [
        {
            "x": x[b],
            "w_attn": w_attn,
            "b_attn": b_attn,
            "w_proj": w_proj,
            "b_proj": b_proj,
            "cst": cst,
        }
        for b in range(B)
    ]
    res = run_bass_kernel_spmd(nc, in_maps, list(range(B)))
    return np.stack([res.results[b]["out"] for b in range(B)], axis=0).astype(np.float32)


# revision 18
# speedup vs baseline: 1.1797x; 1.0995x over previous
"""Causal multi-head attention block (qkv -> causal softmax(qk^T/sqrt(d)) v -> proj)
for Trainium2, batch-sharded across 8 NeuronCores (1 batch element per core).

Self-contained: hardcodes shapes B=8, S=1024, NX=1024, H=16, D=64.

Per-core dataflow (all matmuls fp32r = full-rate reduced-precision fp32):
  x [S,NX] --PE transpose--> xT [NX,S]
  v  = xT.T @ w_v   (+b)    -> v  [S,NX]   (natural)
  qkT = w_qk.T...: matmul(lhsT=w_attn[:, :2048], rhs=xT) -> qT,kT [2048,S] (transposed)
  per head pair (2 heads/128 partitions):
    sT[k,q] = matmul(lhsT=kT_h[64,k-chunk], rhs=qT_h[64,q-chunk])   row-tiled x2 heads
    causal:   fully-masked tiles skipped; diagonal 128x128 blocks get
              += I.T @ T  (T = -30000 below diagonal) accumulated in PSUM
    wT = Exp(sT/8)                       (ScalarE, PSUM->SBUF fp32r)
    aT_u = matmul(lhsT=v_h[k,64], rhs=wT) col-tiled x2 heads -> [128(d),q]
    sum  = matmul(lhsT=ones[128,1], rhs=wT) col-tiled x2 heads (M=1 each)
    aT   = aT_u * broadcast(1/sum)       (VectorE eviction, fp32r)
  out = matmul(lhsT=aT, rhs=w_proj) (+b) -> [S,NX]
"""

import sys

if "/opt/trn_rl_repo" not in sys.path:
    sys.path.insert(0, "/opt/trn_rl_repo")

import numpy as np
from contextlib import ExitStack

import concourse.bass as bass
import concourse.mybir as mybir
import concourse.tile as tile
from concourse import bacc
from concourse.bass_utils import run_bass_kernel_spmd
from concourse.masks import make_identity

F32 = mybir.dt.float32
F32R = mybir.dt.float32r
BF16 = mybir.dt.bfloat16
EXP = mybir.ActivationFunctionType.Exp

P = 128
S = 1024
NX = 1024
H = 16
D = 64
NT = S // P            # 8 partition tiles along S or NX
NQC = 2                # 512-wide q windows
QW = 512
NPAIR = H // 2         # 8 head pairs
SCALE = 0.125          # 1/sqrt(64)
NEGBIG = -30000.0      # pre-scale mask add; exp(SCALE*NEGBIG) == 0 in fp32


def build_nc(with_bias=False):
    nc = bacc.Bacc("TRN2", target_bir_lowering=False, debug=False)

    x_d = nc.dram_tensor("x", [S, NX], F32R, kind="ExternalInput").ap()
    wattn_d = nc.dram_tensor("w_attn", [NX, 3 * NX], F32R, kind="ExternalInput").ap()
    battn_d = nc.dram_tensor("b_attn", [3 * NX], F32R, kind="ExternalInput").ap()
    wproj_d = nc.dram_tensor("w_proj", [NX, NX], F32R, kind="ExternalInput").ap()
    bproj_d = nc.dram_tensor("b_proj", [NX], F32R, kind="ExternalInput").ap()
    cst_d = nc.dram_tensor("cst", [P, 2 * NX], F32R, kind="ExternalInput").ap()
    out_d = nc.dram_tensor("out", [S, NX], F32, kind="ExternalOutput").ap()

    with tile.TileContext(nc) as tc, ExitStack() as ctx:
        const = ctx.enter_context(tc.tile_pool(name="const", bufs=1))
        p_xT = ctx.enter_context(tc.tile_pool(name="p_xT", bufs=1))
        p_v = ctx.enter_context(tc.tile_pool(name="p_v", bufs=1))
        p_aT = ctx.enter_context(tc.tile_pool(name="p_aT", bufs=1))

        # ---------- constants ----------
        ident32 = const.tile([P, P], F32R)
        nc.sync.dma_start(ident32[:], cst_d[:, QW : QW + P])
        identb = const.tile([P, P], BF16)
        make_identity(nc, identb)
        tpat = const.tile([P, P], BF16)
        nc.gpsimd.memset(tpat[:], 0.0)
        # tpat[r, c] = 0 if r <= c else NEGBIG   (k on partitions, q on free)
        nc.gpsimd.affine_select(
            out=tpat[:], in_=tpat[:],
            compare_op=mybir.AluOpType.is_ge,
            fill=NEGBIG, base=0,
            pattern=[[1, P]], channel_multiplier=-1,
        )
        ones_row = const.tile([1, QW], F32R)
        nc.sync.dma_start(ones_row[:], cst_d[0:1, 0:QW])
        battn_sb = const.tile([1, 2 * NX], F32R)
        nc.sync.dma_start(battn_sb[:], battn_d.rearrange("(a b) -> a b", a=1)[:, 0 : 2 * NX])

        xT = [p_xT.tile([P, S], F32R, name=f"xT{i}") for i in range(NT)]

        # ---------- phase 1: x transpose ----------
        with tc.tile_pool(name="p_x", bufs=3) as p_x, \
             tc.tile_pool(name="ps_tr", bufs=2, space="PSUM") as ps_tr:
            for nt in range(NT):
                # column block nt of x, all row tiles: feeds xT[nt] completely
                xt_in = p_x.tile([P, NT, P], F32R, name="xin")
                nc.sync.dma_start(
                    xt_in[:],
                    x_d[:, nt * P : (nt + 1) * P].rearrange("(st p) c -> p st c", p=P),
                )
                for st in range(NT):
                    pst = ps_tr.tile([P, P], F32R, name="trp")
                    nc.tensor.transpose(pst[:], xt_in[:, st, :], ident32[:])
                    nc.vector.tensor_copy(xT[nt][:, st * P : (st + 1) * P], pst[:])

        # ---------- phase 2: v = x @ w_v + b_v (natural [S, NX]), augmented with a
        # ones column per head: v_aug[s, h, 0:64] = v head h, v_aug[s, h, 64] = 1
        v = [p_v.tile([P, H, D + 1], F32R, name=f"v{i}") for i in range(NT)]
        with tc.tile_pool(name="p_wv", bufs=1) as p_wv, \
             tc.tile_pool(name="ps_v", bufs=2, space="PSUM") as ps_v:
            battn_v = p_wv.tile([1, NX], F32R, name="battn_v")
            nc.sync.dma_start(
                battn_v[:], battn_d.rearrange("(a b) -> a b", a=1)[:, 2 * NX : 3 * NX]
            )
            wv = [p_wv.tile([P, NX], F32R, name=f"wv{i}") for i in range(NT)]
            for kt in range(NT):
                nc.sync.dma_start(wv[kt][:], wattn_d[kt * P : (kt + 1) * P, 2 * NX : 3 * NX])
            for mt in range(NT):
                nc.sync.dma_start(v[mt][:, :, D : D + 1], cst_d[:, 0:H])
            for mt in range(NT):
                for nn in range(NQC):
                    ps = ps_v.tile([P, QW], F32, name="vps")
                    for kt in range(NT):
                        nc.tensor.matmul(
                            ps[:], xT[kt][:, mt * P : (mt + 1) * P],
                            wv[kt][:, nn * QW : (nn + 1) * QW],
                            start=(kt == 0), stop=(kt == NT - 1 and not with_bias),
                        )
                    if with_bias:
                        nc.tensor.matmul(
                            ps[:], ones_row[:, 0:P],
                            battn_v[:, nn * QW : (nn + 1) * QW],
                            start=False, stop=True,
                        )
                    nc.vector.tensor_copy(
                        v[mt][:, nn * (QW // D) : (nn + 1) * (QW // D), 0:D],
                        ps[:].rearrange("p (h d) -> p h d", d=D),
                    )

        # ---------- phases 3+4: per-pair qkT then attention ----------
        aT = [p_aT.tile([P, S], F32R, name=f"aT{i}") for i in range(NT)]

        attn_ctx = ExitStack()
        p_qk = attn_ctx.enter_context(tc.tile_pool(name="p_qk", bufs=2))
        p_wT = attn_ctx.enter_context(tc.tile_pool(name="p_wT", bufs=1))
        p_wqk = attn_ctx.enter_context(tc.tile_pool(name="p_wqk", bufs=3))
        p_misc = attn_ctx.enter_context(tc.tile_pool(name="p_misc", bufs=2))
        p_bc = attn_ctx.enter_context(tc.tile_pool(name="p_bc", bufs=2))
        wT_A = [p_wT.tile([P, S], F32R, name=f"wTA{i}") for i in range(NT)]
        wT_B = [p_wT.tile([P, S], F32R, name=f"wTB{i}") for i in range(NT)]
        # dead (fully-masked) column ranges zeroed once; exp evictions never touch them
        for ki in range(1, NT):
            nc.sync.dma_start(wT_A[ki][:, 0 : ki * P], cst_d[:, NX : NX + ki * P])
            nc.sync.dma_start(wT_B[ki][:, 0 : ki * P], cst_d[:, NX : NX + ki * P])

        ps_qk = attn_ctx.enter_context(tc.tile_pool(name="ps_qk", bufs=2, space="PSUM"))
        ps_sc = attn_ctx.enter_context(tc.tile_pool(name="ps_sc", bufs=2, space="PSUM"))
        ps_av = attn_ctx.enter_context(tc.tile_pool(name="ps_av", bufs=2, space="PSUM"))

        def emit_qkT(t):
            # qkT for pair t: M-tiles t (q rows) and 8+t (k rows)
            pair_tiles = []
            for idx, which in enumerate((t, NPAIR + t)):
                dst = p_qk.tile([P, S], F32R, name=("qpair" if idx == 0 else "kpair"))
                wq = p_wqk.tile([P, NT, P], F32R, name="wqk")
                nc.sync.dma_start(
                    wq[:],
                    wattn_d[:, which * P : (which + 1) * P].rearrange(
                        "(kt p) c -> p kt c", p=P
                    ),
                )
                for nn in range(NQC):
                    ps = ps_qk.tile([P, QW], F32, name="qkps")
                    for kt in range(NT):
                        nc.tensor.matmul(
                            ps[:], wq[:, kt, :], xT[kt][:, nn * QW : (nn + 1) * QW],
                            start=(kt == 0), stop=(kt == NT - 1 and not with_bias),
                        )
                    if with_bias:
                        nc.tensor.matmul(
                            ps[:], battn_sb[:, which * P : (which + 1) * P],
                            ones_row[:],
                            start=False, stop=True,
                        )
                    nc.scalar.copy(dst[:, nn * QW : (nn + 1) * QW], ps[:])
                pair_tiles.append(dst)
            return pair_tiles

        def emit_scores_exp(t, q_pair, k_pair):
            for qc in range(NQC):
                live = range(0, 4 if qc == 0 else NT)
                for ki in live:
                    has_diag = (qc * QW) <= ki * P < (qc + 1) * QW
                    offd = ki * P - qc * QW
                    off0 = max(0, offd)
                    nmm = max(256, QW - off0)   # fp32r needs N>=256 for full rate
                    offm = QW - nmm
                    sts = []
                    for h, (hb, tp) in enumerate(((0, (0, 0)), (64, (64, 0)))):
                        stt = ps_sc.tile([P, QW], F32, name=("sta" if h == 0 else "stb"))
                        nc.tensor.matmul(
                            stt[:, offm:QW],
                            k_pair[hb : hb + 64, ki * P : (ki + 1) * P],
                            q_pair[hb : hb + 64, qc * QW + offm : (qc + 1) * QW],
                            start=True, stop=not has_diag,
                            tile_position=tp,
                        )
                        sts.append(stt)
                    if has_diag:
                        for stt in sts:
                            nc.tensor.matmul(
                                stt[:, offd : offd + P], identb[:], tpat[:],
                                start=False, stop=True,
                            )
                    for stt, wTh in ((sts[0], wT_A), (sts[1], wT_B)):
                        nc.scalar.activation(
                            wTh[ki][:, qc * QW + off0 : (qc + 1) * QW],
                            stt[:, off0:QW],
                            EXP, scale=SCALE,
                        )

        def emit_attnv(t):
            for qc in range(NQC):
                live = range(0, 4 if qc == 0 else NT)
                nlive = 4 if qc == 0 else NT
                for h, wTh in ((0, wT_A), (1, wT_B)):
                    av = ps_av.tile([D + 1, QW], F32, name="av")
                    for ki in live:
                        off0 = max(0, ki * P - qc * QW)
                        offm = QW - max(256, QW - off0)
                        nc.tensor.matmul(
                            av[:, offm:QW], v[ki][:, 2 * t + h, :],
                            wTh[ki][:, qc * QW + offm : (qc + 1) * QW],
                            start=(ki == 0), stop=(ki == nlive - 1),
                        )
                    rec = p_misc.tile([1, QW], F32, name="rec")
                    nc.vector.reciprocal(rec[0:1, :], av[D : D + 1, :])
                    bcast = p_bc.tile([D, QW], F32, name="bcast")
                    nc.gpsimd.partition_broadcast(bcast[:, :], rec[0:1, :], channels=D)
                    nc.vector.tensor_mul(
                        aT[t][h * D : (h + 1) * D, qc * QW : (qc + 1) * QW],
                        av[0:D, :], bcast[:],
                    )

        # software pipeline: qkT(t+1) is emitted between scores/exp(t) and
        # attn x V(t), so the PE chews on qkT matmuls while ScalarE drains exps
        cur = emit_qkT(0)
        for t in range(NPAIR):
            emit_scores_exp(t, cur[0], cur[1])
            nxt = emit_qkT(t + 1) if t + 1 < NPAIR else None
            emit_attnv(t)
            cur = nxt

        attn_ctx.close()

        # ---------- phase 5: out = a @ w_proj + b ----------
        with tc.tile_pool(name="p_wp", bufs=1) as p_wp, \
             tc.tile_pool(name="p_out", bufs=2) as p_out, \
             tc.tile_pool(name="ps_o", bufs=2, space="PSUM") as ps_o:
            bproj_sb = p_wp.tile([1, NX], F32R, name="bproj")
            nc.sync.dma_start(bproj_sb[:], bproj_d.rearrange("(a b) -> a b", a=1))
            wp = [p_wp.tile([P, NX], F32R, name=f"wp{i}") for i in range(NT)]
            for kt in range(NT):
                nc.sync.dma_start(wp[kt][:], wproj_d[kt * P : (kt + 1) * P, :])
            for mt in range(NT):
                ot = p_out.tile([P, NX], F32, name="ot")
                for nn in range(NQC):
                    ps = ps_o.tile([P, QW], F32, name="ops")
                    for kt in range(NT):
                        nc.tensor.matmul(
                            ps[:], aT[kt][:, mt * P : (mt + 1) * P],
                            wp[kt][:, nn * QW : (nn + 1) * QW],
                            start=(kt == 0), stop=(kt == NT - 1 and not with_bias),
                        )
                    if with_bias:
                        nc.tensor.matmul(
                            ps[:], ones_row[:, 0:P],
                            bproj_sb[:, nn * QW : (nn + 1) * QW],
                            start=False, stop=True,
                        )
                    nc.vector.tensor_copy(ot[:, nn * QW : (nn + 1) * QW], ps[:])
                nc.sync.dma_start(out_d[mt * P : (mt + 1) * P, :], ot[:])

    nc.compile()
    return nc


_NC_CACHE = {}


def _get_nc(with_bias=False):
    key = "nc_bias" if with_bias else "nc"
    if key not in _NC_CACHE:
        _NC_CACHE[key] = build_nc(with_bias)
    return _NC_CACHE[key]


def _cst_array():
    cst = np.zeros((P, 2 * NX), dtype=np.float32)
    cst[:, :QW] = 1.0
    cst[:, QW : QW + P] = np.eye(P, dtype=np.float32)
    return cst


def kernel(x, w_attn, b_attn, w_proj, b_proj):
    x = np.asarray(x, dtype=np.float32)
    w_attn = np.asarray(w_attn, dtype=np.float32)
    b_attn = np.asarray(b_attn, dtype=np.float32)
    w_proj = np.asarray(w_proj, dtype=np.float32)
    b_proj = np.asarray(b_proj, dtype=np.float32)

    with_bias = bool(np.any(b_attn) or np.any(b_proj))
    nc = _get_nc(with_bias)
    cst = _cst_array()
    B = x.shape[0]
    in_maps = [
        {
            "x": x[b],
            "w_attn": w_attn,
            "b_attn": b_attn,
            "w_proj": w_proj,
            "b_proj": b_proj,
            "cst": cst,
        }
        for b in range(B)
    ]
    res = run_bass_kernel_spmd(nc, in_maps, list(range(B)))
    return np.stack([res.results[b]["out"] for b in range(B)], axis=0).astype(np.float32)
